# revision 16
# baseline (speedup 1.0000x reference)
"""EntropyInvarianceAttention Trainium2 Bass kernel.

Full inputs: q,k,v (4, 512, 2048) f32, k_length (4,) int32.
out = softmax_k(s_b * q^T k) @ v per (b, h) pair, s_b = log(k_length_b)/(8*log 20).

Sharding: 32 (b,h) pairs -> 8 cores x 4 pairs (core c: batch c//2, heads 4*(c%2)..+4).
All 4 pairs on a core share one batch -> one softmax scale s per core
(ln(k_length) is computed on host and passed as the "kl" scalar input).

Per-core kernel, ACT(exp)-bound design:
  - pairs processed in 2 groups of 2 (A,B); K/Q/V for a group live as
    (128, 2048) tiles with pair A on partitions 0-63, B on 64-127
  - S^T = K^T Q per (q-chunk 512, k-tile 128): TWO row-tiled matmuls
    (K=64 contraction each, tile_position (0,0)/(64,0) auto-derived from
    base partitions) run CONCURRENTLY on the PE, writing one
    (128, 1024) f32 PSUM tile (2 banks: A cols 0-511 -> bank a,
    B cols 512-1023 -> bank a+1)
  - ONE ACT instruction per k-tile: P = exp(s*S - 20) over the fused
    (128, 1024) PSUM tile -> SBUF bf16 (shift cancels in normalization)
  - AV: per pair one N=512 matmul accumulating into a (65, 512) 1-bank
    PSUM acc over the 16 k-tiles; lhsT = [V^T | 2^-32] (ones column gives
    the denominator, V pre-scaled by 2^-32 so normalization is exact)
  - PSUM: 2 x 2-bank S tiles (double buffer) + 2 x 2 x 1-bank accs
    (double-buffered across segments) = 8 banks
  - software-pipelined emission (AV lags S/ACT by one k-tile) so the PE
    never head-of-line blocks the ACT stream; ACT is the bottleneck at
    ~1us per 1024-col exp
  - normalize per (group, chunk): DVE copy acc->SBUF, DMA-bounce the
    denominator row through DRAM for partition broadcast, reciprocal +
    multiply on DVE, DMA out
"""
import sys
import numpy as np
from contextlib import ExitStack

sys.path.insert(0, "/opt/trn_rl_repo")

import concourse.bass as bass
import concourse.tile as tile
from concourse import bacc, mybir
from concourse.bass_utils import run_bass_kernel_spmd

F32 = mybir.dt.float32
F16 = mybir.dt.float16
BF16 = mybir.dt.bfloat16
AF = mybir.ActivationFunctionType

B, H, D, L = 4, 8, 64, 2048
N_CORES = 8
PAIRS = 4                 # (b,h) pairs per core
GROUPS = 2                # pair-groups of 2
ROWS = PAIRS * D          # 256 rows of q/k/v per core
KT = L // 128             # 16 k-tiles
QC = L // 512             # 4 q-chunks of 512
SCALE = 1.0 / (D ** 0.5 * float(np.log(20.0)))
C_SHIFT = 20.0            # exp(s*score - C); cancels in softmax normalization
VSCALE = 2.0 ** -32       # carried by numerator AND denominator; cancels


def _emit_prep(nc, pools, g, aps, critical=False):
    """Loads + conversions + V^T construction for pair-group g (pairs 2g, 2g+1).

    critical=True (group 0): chunked loads + parallel DMA queues so the first
    S matmuls can start ~4us in instead of waiting for whole-tensor loads.
    Returns (k16, q16, vtA, vtB)."""
    io, cv, vtp = pools["io"], pools["cv"], pools["vt"]
    r0 = g * 128

    k32 = io.tile([128, L], F32, tag="k32")
    q32 = io.tile([128, L], F32, tag="q32")
    v32 = io.tile([128, L], F32, tag="v32")
    k16 = cv.tile([128, L], F16, tag="k16")
    q16 = cv.tile([128, L], F16, tag="q16")
    v16 = cv.tile([128, L], BF16, tag="v16")

    if critical:
        hk = L // 2
        # v on the gpsimd SWDGE queue, in parallel with k/q on sync
        nc.gpsimd.dma_start(out=v32[:], in_=aps["v"][r0:r0 + 128, :])
        nc.sync.dma_start(out=k32[:, 0:hk], in_=aps["k"][r0:r0 + 128, 0:hk])
        nc.sync.dma_start(out=q32[:, 0:512], in_=aps["q"][r0:r0 + 128, 0:512])
        nc.sync.dma_start(out=k32[:, hk:L], in_=aps["k"][r0:r0 + 128, hk:L])
        nc.sync.dma_start(out=q32[:, 512:L], in_=aps["q"][r0:r0 + 128, 512:L])
        nc.vector.tensor_copy(out=k16[:, 0:hk], in_=k32[:, 0:hk])
        nc.vector.tensor_copy(out=q16[:, 0:512], in_=q32[:, 0:512])
        nc.vector.tensor_scalar_mul(out=v16[:], in0=v32[:], scalar1=VSCALE)
        nc.vector.tensor_copy(out=k16[:, hk:L], in_=k32[:, hk:L])
        nc.vector.tensor_copy(out=q16[:, 512:L], in_=q32[:, 512:L])
    else:
        # prefetch group: everything via gpsimd SWDGE, off the sync queue
        nc.gpsimd.dma_start(out=k32[:], in_=aps["k"][r0:r0 + 128, :])
        nc.gpsimd.dma_start(out=q32[:], in_=aps["q"][r0:r0 + 128, :])
        nc.gpsimd.dma_start(out=v32[:], in_=aps["v"][r0:r0 + 128, :])
        nc.vector.tensor_copy(out=k16[:], in_=k32[:])
        nc.vector.tensor_copy(out=q16[:], in_=q32[:])
        nc.vector.tensor_scalar_mul(out=v16[:], in0=v32[:], scalar1=VSCALE)

    vtA = vtp.tile([128, KT, 128], BF16, tag="vtA")
    vtB = vtp.tile([128, KT, 128], BF16, tag="vtB")
    nc.vector.memset(vtA[:, :, D:D + 1], VSCALE)   # denominator column
    nc.vector.memset(vtB[:, :, D:D + 1], VSCALE)
    # single-instruction SBUF->SBUF XBAR transposes: vt[p, t, d] = v16[d, 128t+p]
    nc.sync.dma_start(out=vtA[:, :, 0:D], in_=v16[0:64, :], transpose=True)
    nc.sync.dma_start(out=vtB[:, :, 0:D], in_=v16[64:128, :], transpose=True)
    return k16, q16, vtA, vtB


def _emit_prep_staged(nc, pools, g, aps):
    """Prefetch-group loads now; returns a dict of stage callbacks keyed by
    relative iteration, so the big DVE conversions are spread across the
    steady-state loop instead of head-of-line blocking the DVE exp tiles."""
    io, cv, vtp = pools["io"], pools["cv"], pools["vt"]
    r0 = g * 128
    k32 = io.tile([128, L], F32, tag="k32")
    q32 = io.tile([128, L], F32, tag="q32")
    v32 = io.tile([128, L], F32, tag="v32")
    k16 = cv.tile([128, L], F16, tag="k16")
    q16 = cv.tile([128, L], F16, tag="q16")
    v16 = cv.tile([128, L], BF16, tag="v16")
    nc.gpsimd.dma_start(out=k32[:], in_=aps["k"][r0:r0 + 128, :])
    nc.gpsimd.dma_start(out=q32[:], in_=aps["q"][r0:r0 + 128, :])
    nc.gpsimd.dma_start(out=v32[:], in_=aps["v"][r0:r0 + 128, :])
    vtA = vtp.tile([128, KT, 128], BF16, tag="vtA")
    vtB = vtp.tile([128, KT, 128], BF16, tag="vtB")
    hk = L // 2
    stages = {
        0: lambda: nc.vector.tensor_copy(out=k16[:, 0:hk], in_=k32[:, 0:hk]),
        3: lambda: nc.vector.tensor_copy(out=k16[:, hk:L], in_=k32[:, hk:L]),
        7: lambda: nc.vector.tensor_copy(out=q16[:, 0:hk], in_=q32[:, 0:hk]),
        10: lambda: nc.vector.tensor_copy(out=q16[:, hk:L], in_=q32[:, hk:L]),
        13: lambda: nc.vector.tensor_scalar_mul(out=v16[:, 0:hk],
                                                in0=v32[:, 0:hk], scalar1=VSCALE),
        16: lambda: nc.vector.tensor_scalar_mul(out=v16[:, hk:L],
                                                in0=v32[:, hk:L], scalar1=VSCALE),
        19: lambda: (nc.vector.memset(vtA[:, :, D:D + 1], VSCALE),
                     nc.vector.memset(vtB[:, :, D:D + 1], VSCALE),
                     nc.sync.dma_start(out=vtA[:, :, 0:D], in_=v16[0:64, :],
                                       transpose=True),
                     nc.sync.dma_start(out=vtB[:, :, 0:D], in_=v16[64:128, :],
                                       transpose=True)),
    }
    return (k16, q16, vtA, vtB), stages


def _emit_norm(nc, pools, seg, aps):
    """Normalize one segment (group g, q-chunk c): two (65, 512) PSUM accs."""
    npo, outp = pools["np"], pools["out"]
    g, c, accA, accB = seg
    csl = slice(512 * c, 512 * (c + 1))
    for half, acc in ((0, accA), (1, accB)):
        p = 2 * g + half
        # evacuate acc (frees its PSUM bank for the next segment)
        av = npo.tile([D + 1, 512], F32, tag="av")
        nc.vector.tensor_copy(out=av[:], in_=acc[:])
        # denominator row -> DRAM -> stride-0 partition broadcast back
        ri = QC * p + c
        nc.sync.dma_start(out=aps["rec"][ri:ri + 1, :], in_=av[D:D + 1, :])
        bc = npo.tile([D, 512], F32, tag="bc")
        rsrc = aps["rec"][ri:ri + 1, :]
        rb = bass.AP(tensor=rsrc.tensor, offset=rsrc.offset,
                     ap=[[0, D]] + [list(a) for a in rsrc.ap[1:]])
        nc.sync.dma_start(out=bc[:], in_=rb)
        rcp = npo.tile([D, 512], F32, tag="rcp")
        nc.vector.reciprocal_approx_fast(out=rcp[:], in_=bc[:])
        out_t = outp.tile([D, 512], F32, tag="out")
        nc.vector.tensor_mul(out=out_t[:], in0=av[0:D, :], in1=rcp[:])
        nc.sync.dma_start(out=aps["out"][64 * p:64 * (p + 1), csl], in_=out_t[:])


def _emit_norm_pe(nc, pools, seg, aps, ones64, ps):
    """Tail-latency normalize for the final segment: broadcast the reciprocal
    denominator across partitions with a K=1 f32 matmul into a free PSUM bank
    instead of the DRAM round trip (saves ~4us of exposed DMA latency)."""
    npo, outp = pools["np"], pools["out"]
    g, c, accA, accB = seg
    csl = slice(512 * c, 512 * (c + 1))
    for half, acc in ((0, accA), (1, accB)):
        p = 2 * g + half
        av = npo.tile([D + 1, 512], F32, tag="av")
        nc.vector.tensor_copy(out=av[:], in_=acc[:])
        rcp = npo.tile([D + 1, 512], F32, tag="rcpr")
        nc.vector.reciprocal_approx_fast(out=rcp[D:D + 1, :], in_=av[D:D + 1, :])
        bcp = ps.tile([D, 512], F32, tag="sc", bufs=2, name=f"bcp_{half}")
        nc.tensor.matmul(out=bcp[:], lhsT=ones64[D:D + 1, :],
                         rhs=rcp[D:D + 1, :], start=True, stop=True)
        out_t = outp.tile([D, 512], F32, tag="out")
        nc.vector.tensor_mul(out=out_t[:], in0=av[0:D, :], in1=bcp[:])
        nc.sync.dma_start(out=aps["out"][64 * p:64 * (p + 1), csl], in_=out_t[:])


def build():
    nc = bacc.Bacc("TRN2", target_bir_lowering=False, debug=False)
    aps = {
        "q": nc.dram_tensor("q", [ROWS, L], F32, kind="ExternalInput").ap(),
        "k": nc.dram_tensor("k", [ROWS, L], F32, kind="ExternalInput").ap(),
        "v": nc.dram_tensor("v", [ROWS, L], F32, kind="ExternalInput").ap(),
        "kl": nc.dram_tensor("kl", [1, 1], F32, kind="ExternalInput").ap(),
        "out": nc.dram_tensor("out", [ROWS, L], F32, kind="ExternalOutput").ap(),
        "rec": nc.dram_tensor("recs", [PAIRS * QC, 512], F32).ap(),  # scratch
    }

    with tile.TileContext(nc) as tc, ExitStack() as ctx:
        pools = {
            "io": ctx.enter_context(tc.tile_pool(name="io", bufs=2)),
            "cv": ctx.enter_context(tc.tile_pool(name="cv", bufs=2)),
            "vt": ctx.enter_context(tc.tile_pool(name="vt", bufs=2)),
            "pt": ctx.enter_context(tc.tile_pool(name="pt", bufs=4)),
            "np": ctx.enter_context(tc.tile_pool(name="np", bufs=3)),
            "out": ctx.enter_context(tc.tile_pool(name="out", bufs=3)),
            "cst": ctx.enter_context(tc.tile_pool(name="cst", bufs=1)),
            "ps": ctx.enter_context(tc.tile_pool(name="ps", bufs=1, space="PSUM")),
        }
        cst, ps, pt = pools["cst"], pools["ps"], pools["pt"]

        # warm the exp table-set load (~2.6us) during the input DMAs
        warm = cst.tile([128, 1], F32)
        nc.vector.memset(warm[:], 0.0)
        nc.scalar.activation(out=warm[:], in_=warm[:], func=AF.Exp)

        # s = SCALE * ln(k_length) (host passes ln(k_length) in kl)
        kl_b = cst.tile([128, 1], F32)
        kl_bcast = bass.AP(tensor=aps["kl"].tensor, offset=aps["kl"].offset,
                           ap=[[0, 128], [1, 1]])
        nc.sync.dma_start(out=kl_b[:], in_=kl_bcast)
        s128 = cst.tile([128, 1], F32)
        nc.vector.tensor_scalar_mul(out=s128[:], in0=kl_b[:], scalar1=SCALE)
        negc = cst.tile([128, 1], F32)
        nc.vector.memset(negc[:], -C_SHIFT)
        # Schraudolph-to-bf16 exp on DVE: bf16_bits(exp(s*S - 20)) ~=
        # int16(S * (128*log2e*s) + (16256 - 2560*log2e - CC)); the int16
        # result is written straight into the bf16 ptk tile via bitcast.
        LOG2E = 1.4426950408889634
        SCH_CC = 6.0
        a128 = cst.tile([128, 1], F32)
        nc.vector.tensor_scalar_mul(out=a128[:], in0=s128[:],
                                    scalar1=128.0 * LOG2E)
        SCH_B = 16256.0 - 2560.0 * LOG2E - SCH_CC

        prep = {0: _emit_prep(nc, pools, 0, aps, critical=True)}
        stages = {}
        ITERS = GROUPS * QC * KT       # 128
        pend_av = None                 # (ptk, t, vtA, vtB)
        pend_norm = None               # (g, c, accA, accB)
        accA = accB = None
        for i in range(ITERS):
            g, c, t = i // (QC * KT), (i // KT) % QC, i % KT
            k16, q16, vtA, vtB = prep[g]
            csl = slice(512 * c, 512 * (c + 1))

            # S^T for k-tile t, both pairs, row-tiled concurrent matmuls
            sc = ps.tile([128, 1024], F32, tag="sc", bufs=2)
            nc.tensor.matmul(out=sc[:, 0:512],
                             lhsT=k16[0:64, 128 * t:128 * (t + 1)],
                             rhs=q16[0:64, csl], start=True, stop=True)
            nc.tensor.matmul(out=sc[:, 512:1024],
                             lhsT=k16[64:128, 128 * t:128 * (t + 1)],
                             rhs=q16[64:128, csl], start=True, stop=True)

            # P^T = exp(s*S - C) for both pairs: ACT for 2/3 of the k-tiles,
            # DVE Schraudolph (one tensor_scalar into the int16 view) for the
            # rest, so the exp work is split across both engines
            # (a DVE Schraudolph offload via ptk.bitcast(int16) was tried here:
            # device semantics verified correct in isolation, but bitcast APs
            # lose the Tile identity, so the write lowers to a phantom tensor
            # outside dependency tracking -> stale reads; reverted)
            ptk = pt.tile([128, 1024], BF16, tag="ptk")
            nc.scalar.activation(out=ptk[:], in_=sc[:], func=AF.Exp,
                                 bias=negc[:], scale=s128[:])

            if i == 16:
                prep[1], st = _emit_prep_staged(nc, pools, 1, aps)
                stages = {16 + 2 + k: v for k, v in st.items()}
            if i in stages:
                stages[i]()

            # AV for the previous k-tile (lag 1 so the PE stream never
            # head-of-line blocks behind the ACT result)
            if pend_av is not None:
                pptk, pt_t, pvtA, pvtB = pend_av
                if pt_t == 0:
                    if pend_norm is not None:
                        _emit_norm(nc, pools, pend_norm, aps)
                    pg, pc = (i - 1) // (QC * KT), ((i - 1) // KT) % QC
                    accA = ps.tile([D + 1, 512], F32, tag="avA", bufs=2,
                                   name=f"accA_{pg}_{pc}")
                    accB = ps.tile([D + 1, 512], F32, tag="avB", bufs=2,
                                   name=f"accB_{pg}_{pc}")
                    pend_norm = (pg, pc, accA, accB)
                nc.tensor.matmul(out=accA[:], lhsT=pvtA[:, pt_t, 0:D + 1],
                                 rhs=pptk[:, 0:512],
                                 start=(pt_t == 0), stop=(pt_t == KT - 1),
                                 skip_group_check=True)
                nc.tensor.matmul(out=accB[:], lhsT=pvtB[:, pt_t, 0:D + 1],
                                 rhs=pptk[:, 512:1024],
                                 start=(pt_t == 0), stop=(pt_t == KT - 1),
                                 skip_group_check=True)
            pend_av = (ptk, t, vtA, vtB)

        # drain: AV for the last k-tile, then the last two norms
        pptk, pt_t, pvtA, pvtB = pend_av
        nc.tensor.matmul(out=accA[:], lhsT=pvtA[:, pt_t, 0:D + 1],
                         rhs=pptk[:, 0:512], start=False, stop=True,
                         skip_group_check=True)
        nc.tensor.matmul(out=accB[:], lhsT=pvtB[:, pt_t, 0:D + 1],
                         rhs=pptk[:, 512:1024], start=False, stop=True,
                         skip_group_check=True)
        _emit_norm(nc, pools, pend_norm, aps)

    nc.compile()
    return nc


_NC = None


def _get_nc():
    global _NC
    if _NC is None:
        _NC = build()
    return _NC


def make_in_maps(q, k, v, k_length):
    q = np.ascontiguousarray(q, dtype=np.float32)
    k = np.ascontiguousarray(k, dtype=np.float32)
    v = np.ascontiguousarray(v, dtype=np.float32)
    k_length = np.asarray(k_length)
    in_maps = []
    for c in range(N_CORES):
        b = c // 2
        r0 = (c % 2) * ROWS
        in_maps.append({
            "q": np.ascontiguousarray(q[b, r0:r0 + ROWS, :]),
            "k": np.ascontiguousarray(k[b, r0:r0 + ROWS, :]),
            "v": np.ascontiguousarray(v[b, r0:r0 + ROWS, :]),
            "kl": np.array([[np.log(np.float32(k_length[b]))]],
                           dtype=np.float32),
        })
    return in_maps


def kernel(q, k, v, k_length):
    in_maps = make_in_maps(q, k, v, k_length)
    nc = _get_nc()
    res = run_bass_kernel_spmd(nc, in_maps, core_ids=list(range(N_CORES)))

    out = np.empty((B, H * D, L), dtype=np.float32)
    for c in range(N_CORES):
        b = c // 2
        r0 = (c % 2) * ROWS
        out[b, r0:r0 + ROWS, :] = res.results[c]["out"]
    return out


# revision 20
# speedup vs baseline: 1.0060x; 1.0060x over previous
"""EntropyInvarianceAttention Trainium2 Bass kernel.

Full inputs: q,k,v (4, 512, 2048) f32, k_length (4,) int32.
out = softmax_k(s_b * q^T k) @ v per (b, h) pair, s_b = log(k_length_b)/(8*log 20).

Sharding: 32 (b,h) pairs -> 8 cores x 4 pairs (core c: batch c//2, heads 4*(c%2)..+4).
All 4 pairs on a core share one batch -> one softmax scale s per core
(ln(k_length) is computed on host and passed as the "kl" scalar input).

Per-core kernel, ACT(exp)-bound design:
  - pairs processed in 2 groups of 2 (A,B); K/Q/V for a group live as
    (128, 2048) tiles with pair A on partitions 0-63, B on 64-127
  - S^T = K^T Q per (q-chunk 512, k-tile 128): TWO row-tiled matmuls
    (K=64 contraction each, tile_position (0,0)/(64,0) auto-derived from
    base partitions) run CONCURRENTLY on the PE, writing one
    (128, 1024) f32 PSUM tile (2 banks: A cols 0-511 -> bank a,
    B cols 512-1023 -> bank a+1)
  - ONE ACT instruction per k-tile: P = exp(s*S - 20) over the fused
    (128, 1024) PSUM tile -> SBUF bf16 (shift cancels in normalization)
  - AV: per pair one N=512 matmul accumulating into a (65, 512) 1-bank
    PSUM acc over the 16 k-tiles; lhsT = [V^T | 2^-32] (ones column gives
    the denominator, V pre-scaled by 2^-32 so normalization is exact)
  - PSUM: 2 x 2-bank S tiles (double buffer) + 2 x 2 x 1-bank accs
    (double-buffered across segments) = 8 banks
  - software-pipelined emission (AV lags S/ACT by one k-tile) so the PE
    never head-of-line blocks the ACT stream; ACT is the bottleneck at
    ~1us per 1024-col exp
  - normalize per (group, chunk): DVE copy acc->SBUF, DMA-bounce the
    denominator row through DRAM for partition broadcast, reciprocal +
    multiply on DVE, DMA out
"""
import sys
import numpy as np
from contextlib import ExitStack

sys.path.insert(0, "/opt/trn_rl_repo")

import concourse.bass as bass
import concourse.tile as tile
from concourse import bacc, mybir
from concourse.bass_utils import run_bass_kernel_spmd

F32 = mybir.dt.float32
F16 = mybir.dt.float16
BF16 = mybir.dt.bfloat16
AF = mybir.ActivationFunctionType

B, H, D, L = 4, 8, 64, 2048
N_CORES = 8
PAIRS = 4                 # (b,h) pairs per core
GROUPS = 2                # pair-groups of 2
ROWS = PAIRS * D          # 256 rows of q/k/v per core
KT = L // 128             # 16 k-tiles
QC = L // 512             # 4 q-chunks of 512
SCALE = 1.0 / (D ** 0.5 * float(np.log(20.0)))
C_SHIFT = 20.0            # exp(s*score - C); cancels in softmax normalization
VSCALE = 2.0 ** -32       # carried by numerator AND denominator; cancels


def _emit_prep(nc, pools, g, aps, critical=False):
    """Loads + conversions + V^T construction for pair-group g (pairs 2g, 2g+1).

    critical=True (group 0): chunked loads + parallel DMA queues so the first
    S matmuls can start ~4us in instead of waiting for whole-tensor loads.
    Returns (k16, q16, vtA, vtB)."""
    io, cv, vtp = pools["io"], pools["cv"], pools["vt"]
    r0 = g * 128

    k32 = io.tile([128, L], F32, tag="k32")
    q32 = io.tile([128, L], F32, tag="q32")
    v32 = io.tile([128, L], F32, tag="v32")
    k16 = cv.tile([128, L], F16, tag="k16")
    q16 = cv.tile([128, L], F16, tag="q16")
    v16 = cv.tile([128, L], BF16, tag="v16")

    if critical:
        hk = L // 2
        # k/q chunks on the fast gpsimd SWDGE queue (~135 GB/s measured vs
        # ~50 on the sync/scalar HWDGE queues); v split across sync+scalar
        # in parallel -- the AV lag of 4 tolerates v arriving later
        nc.gpsimd.dma_start(out=k32[:, 0:hk], in_=aps["k"][r0:r0 + 128, 0:hk])
        nc.gpsimd.dma_start(out=q32[:, 0:512], in_=aps["q"][r0:r0 + 128, 0:512])
        nc.gpsimd.dma_start(out=k32[:, hk:L], in_=aps["k"][r0:r0 + 128, hk:L])
        nc.gpsimd.dma_start(out=q32[:, 512:L], in_=aps["q"][r0:r0 + 128, 512:L])
        nc.sync.dma_start(out=v32[0:64, :], in_=aps["v"][r0:r0 + 64, :])
        nc.scalar.dma_start(out=v32[64:128, :], in_=aps["v"][r0 + 64:r0 + 128, :])
        nc.vector.tensor_copy(out=k16[:, 0:hk], in_=k32[:, 0:hk])
        nc.vector.tensor_copy(out=q16[:, 0:512], in_=q32[:, 0:512])
        nc.vector.tensor_scalar_mul(out=v16[0:64, :], in0=v32[0:64, :],
                                    scalar1=VSCALE)
        nc.vector.tensor_copy(out=k16[:, hk:L], in_=k32[:, hk:L])
        nc.vector.tensor_scalar_mul(out=v16[64:128, :], in0=v32[64:128, :],
                                    scalar1=VSCALE)
        nc.vector.tensor_copy(out=q16[:, 512:L], in_=q32[:, 512:L])
    else:
        # prefetch group: everything via gpsimd SWDGE, off the sync queue
        nc.gpsimd.dma_start(out=k32[:], in_=aps["k"][r0:r0 + 128, :])
        nc.gpsimd.dma_start(out=q32[:], in_=aps["q"][r0:r0 + 128, :])
        nc.gpsimd.dma_start(out=v32[:], in_=aps["v"][r0:r0 + 128, :])
        nc.vector.tensor_copy(out=k16[:], in_=k32[:])
        nc.vector.tensor_copy(out=q16[:], in_=q32[:])
        nc.vector.tensor_scalar_mul(out=v16[:], in0=v32[:], scalar1=VSCALE)

    vtA = vtp.tile([128, KT, 128], BF16, tag="vtA")
    vtB = vtp.tile([128, KT, 128], BF16, tag="vtB")
    nc.vector.memset(vtA[:, :, D:D + 1], VSCALE)   # denominator column
    nc.vector.memset(vtB[:, :, D:D + 1], VSCALE)
    # single-instruction SBUF->SBUF XBAR transposes: vt[p, t, d] = v16[d, 128t+p]
    nc.sync.dma_start(out=vtA[:, :, 0:D], in_=v16[0:64, :], transpose=True)
    nc.sync.dma_start(out=vtB[:, :, 0:D], in_=v16[64:128, :], transpose=True)
    return k16, q16, vtA, vtB


def _emit_prep_staged(nc, pools, g, aps):
    """Prefetch-group loads now; returns a dict of stage callbacks keyed by
    relative iteration, so the big DVE conversions are spread across the
    steady-state loop instead of head-of-line blocking the DVE exp tiles."""
    io, cv, vtp = pools["io"], pools["cv"], pools["vt"]
    r0 = g * 128
    k32 = io.tile([128, L], F32, tag="k32")
    q32 = io.tile([128, L], F32, tag="q32")
    v32 = io.tile([128, L], F32, tag="v32")
    k16 = cv.tile([128, L], F16, tag="k16")
    q16 = cv.tile([128, L], F16, tag="q16")
    v16 = cv.tile([128, L], BF16, tag="v16")
    nc.gpsimd.dma_start(out=k32[:], in_=aps["k"][r0:r0 + 128, :])
    nc.gpsimd.dma_start(out=q32[:], in_=aps["q"][r0:r0 + 128, :])
    nc.gpsimd.dma_start(out=v32[:], in_=aps["v"][r0:r0 + 128, :])
    vtA = vtp.tile([128, KT, 128], BF16, tag="vtA")
    vtB = vtp.tile([128, KT, 128], BF16, tag="vtB")
    hk = L // 2
    stages = {
        0: lambda: nc.vector.tensor_copy(out=k16[:, 0:hk], in_=k32[:, 0:hk]),
        3: lambda: nc.vector.tensor_copy(out=k16[:, hk:L], in_=k32[:, hk:L]),
        7: lambda: nc.vector.tensor_copy(out=q16[:, 0:hk], in_=q32[:, 0:hk]),
        10: lambda: nc.vector.tensor_copy(out=q16[:, hk:L], in_=q32[:, hk:L]),
        13: lambda: nc.vector.tensor_scalar_mul(out=v16[:, 0:hk],
                                                in0=v32[:, 0:hk], scalar1=VSCALE),
        16: lambda: nc.vector.tensor_scalar_mul(out=v16[:, hk:L],
                                                in0=v32[:, hk:L], scalar1=VSCALE),
        19: lambda: (nc.vector.memset(vtA[:, :, D:D + 1], VSCALE),
                     nc.vector.memset(vtB[:, :, D:D + 1], VSCALE),
                     nc.sync.dma_start(out=vtA[:, :, 0:D], in_=v16[0:64, :],
                                       transpose=True),
                     nc.sync.dma_start(out=vtB[:, :, 0:D], in_=v16[64:128, :],
                                       transpose=True)),
    }
    return (k16, q16, vtA, vtB), stages


def _emit_norm(nc, pools, seg, aps):
    """Normalize one segment (group g, q-chunk c): two (65, 512) PSUM accs."""
    npo, outp = pools["np"], pools["out"]
    g, c, accA, accB = seg
    csl = slice(512 * c, 512 * (c + 1))
    for half, acc in ((0, accA), (1, accB)):
        p = 2 * g + half
        # evacuate acc (frees its PSUM bank for the next segment)
        av = npo.tile([D + 1, 512], F32, tag="av")
        nc.vector.tensor_copy(out=av[:], in_=acc[:])
        # denominator row -> DRAM -> stride-0 partition broadcast back
        ri = QC * p + c
        nc.sync.dma_start(out=aps["rec"][ri:ri + 1, :], in_=av[D:D + 1, :])
        bc = npo.tile([D, 512], F32, tag="bc")
        rsrc = aps["rec"][ri:ri + 1, :]
        rb = bass.AP(tensor=rsrc.tensor, offset=rsrc.offset,
                     ap=[[0, D]] + [list(a) for a in rsrc.ap[1:]])
        nc.sync.dma_start(out=bc[:], in_=rb)
        rcp = npo.tile([D, 512], F32, tag="rcp")
        nc.vector.reciprocal_approx_fast(out=rcp[:], in_=bc[:])
        out_t = outp.tile([D, 512], F32, tag="out")
        nc.vector.tensor_mul(out=out_t[:], in0=av[0:D, :], in1=rcp[:])
        nc.sync.dma_start(out=aps["out"][64 * p:64 * (p + 1), csl], in_=out_t[:])


def _emit_norm_pe(nc, pools, seg, aps, ones64, ps):
    """Tail-latency normalize for the final segment: broadcast the reciprocal
    denominator across partitions with a K=1 f32 matmul into a free PSUM bank
    instead of the DRAM round trip (saves ~4us of exposed DMA latency)."""
    npo, outp = pools["np"], pools["out"]
    g, c, accA, accB = seg
    csl = slice(512 * c, 512 * (c + 1))
    for half, acc in ((0, accA), (1, accB)):
        p = 2 * g + half
        av = npo.tile([D + 1, 512], F32, tag="av")
        nc.vector.tensor_copy(out=av[:], in_=acc[:])
        rcp = npo.tile([D + 1, 512], F32, tag="rcpr")
        nc.vector.reciprocal_approx_fast(out=rcp[D:D + 1, :], in_=av[D:D + 1, :])
        bcp = ps.tile([D, 512], F32, tag="sc", bufs=2, name=f"bcp_{half}")
        nc.tensor.matmul(out=bcp[:], lhsT=ones64[D:D + 1, :],
                         rhs=rcp[D:D + 1, :], start=True, stop=True)
        out_t = outp.tile([D, 512], F32, tag="out")
        nc.vector.tensor_mul(out=out_t[:], in0=av[0:D, :], in1=bcp[:])
        nc.sync.dma_start(out=aps["out"][64 * p:64 * (p + 1), csl], in_=out_t[:])


def build():
    nc = bacc.Bacc("TRN2", target_bir_lowering=False, debug=False)
    aps = {
        "q": nc.dram_tensor("q", [ROWS, L], F32, kind="ExternalInput").ap(),
        "k": nc.dram_tensor("k", [ROWS, L], F32, kind="ExternalInput").ap(),
        "v": nc.dram_tensor("v", [ROWS, L], F32, kind="ExternalInput").ap(),
        "kl": nc.dram_tensor("kl", [1, 1], F32, kind="ExternalInput").ap(),
        "out": nc.dram_tensor("out", [ROWS, L], F32, kind="ExternalOutput").ap(),
        "rec": nc.dram_tensor("recs", [PAIRS * QC, 512], F32).ap(),  # scratch
    }

    with tile.TileContext(nc) as tc, ExitStack() as ctx:
        pools = {
            "io": ctx.enter_context(tc.tile_pool(name="io", bufs=2)),
            "cv": ctx.enter_context(tc.tile_pool(name="cv", bufs=2)),
            "vt": ctx.enter_context(tc.tile_pool(name="vt", bufs=2)),
            "pt": ctx.enter_context(tc.tile_pool(name="pt", bufs=6)),
            "np": ctx.enter_context(tc.tile_pool(name="np", bufs=3)),
            "out": ctx.enter_context(tc.tile_pool(name="out", bufs=3)),
            "cst": ctx.enter_context(tc.tile_pool(name="cst", bufs=1)),
            "ps": ctx.enter_context(tc.tile_pool(name="ps", bufs=1, space="PSUM")),
        }
        cst, ps, pt = pools["cst"], pools["ps"], pools["pt"]

        # warm the exp table-set load (~2.6us) during the input DMAs
        warm = cst.tile([128, 1], F32)
        nc.vector.memset(warm[:], 0.0)
        nc.scalar.activation(out=warm[:], in_=warm[:], func=AF.Exp)

        # s = SCALE * ln(k_length) (host passes ln(k_length) in kl)
        kl_b = cst.tile([128, 1], F32)
        kl_bcast = bass.AP(tensor=aps["kl"].tensor, offset=aps["kl"].offset,
                           ap=[[0, 128], [1, 1]])
        nc.sync.dma_start(out=kl_b[:], in_=kl_bcast)
        s128 = cst.tile([128, 1], F32)
        nc.vector.tensor_scalar_mul(out=s128[:], in0=kl_b[:], scalar1=SCALE)
        negc = cst.tile([128, 1], F32)
        nc.vector.memset(negc[:], -C_SHIFT)
        # Schraudolph-to-bf16 exp on DVE: bf16_bits(exp(s*S - 20)) ~=
        # int16(S * (128*log2e*s) + (16256 - 2560*log2e - CC)); the int16
        # result is written straight into the bf16 ptk tile via bitcast.
        LOG2E = 1.4426950408889634
        SCH_CC = 6.0
        a128 = cst.tile([128, 1], F32)
        nc.vector.tensor_scalar_mul(out=a128[:], in0=s128[:],
                                    scalar1=128.0 * LOG2E)
        SCH_B = 16256.0 - 2560.0 * LOG2E - SCH_CC

        prep = {0: _emit_prep(nc, pools, 0, aps, critical=True)}
        stages = {}
        ITERS = GROUPS * QC * KT       # 128
        LAG = 4                        # AV trails S/ACT by LAG k-tiles
        pend_av = []                   # (ptk, t, vtA, vtB, i)
        pend_norm = None               # (g, c, accA, accB)
        accA = accB = None
        for i in range(ITERS):
            g, c, t = i // (QC * KT), (i // KT) % QC, i % KT
            k16, q16, vtA, vtB = prep[g]
            csl = slice(512 * c, 512 * (c + 1))

            # S^T for k-tile t, both pairs, row-tiled concurrent matmuls
            sc = ps.tile([128, 1024], F32, tag="sc", bufs=2)
            nc.tensor.matmul(out=sc[:, 0:512],
                             lhsT=k16[0:64, 128 * t:128 * (t + 1)],
                             rhs=q16[0:64, csl], start=True, stop=True)
            nc.tensor.matmul(out=sc[:, 512:1024],
                             lhsT=k16[64:128, 128 * t:128 * (t + 1)],
                             rhs=q16[64:128, csl], start=True, stop=True)

            # P^T = exp(s*S - C) for both pairs: ACT for 2/3 of the k-tiles,
            # DVE Schraudolph (one tensor_scalar into the int16 view) for the
            # rest, so the exp work is split across both engines
            # (a DVE Schraudolph offload via ptk.bitcast(int16) was tried here:
            # device semantics verified correct in isolation, but bitcast APs
            # lose the Tile identity, so the write lowers to a phantom tensor
            # outside dependency tracking -> stale reads; reverted)
            ptk = pt.tile([128, 1024], BF16, tag="ptk")
            nc.scalar.activation(out=ptk[:], in_=sc[:], func=AF.Exp,
                                 bias=negc[:], scale=s128[:])

            if i == 16:
                prep[1], st = _emit_prep_staged(nc, pools, 1, aps)
                stages = {16 + 2 + k: v for k, v in st.items()}
            if i in stages:
                stages[i]()

            # AV trails by LAG k-tiles so the PE stream never head-of-line
            # blocks behind the ACT result (and V can arrive late at startup)
            pend_av.append((ptk, t, vtA, vtB, i))
            if len(pend_av) > LAG:
                pptk, pt_t, pvtA, pvtB, pi = pend_av.pop(0)
                if pt_t == 0:
                    if pend_norm is not None:
                        _emit_norm(nc, pools, pend_norm, aps)
                    pg, pc = pi // (QC * KT), (pi // KT) % QC
                    accA = ps.tile([D + 1, 512], F32, tag="avA", bufs=2,
                                   name=f"accA_{pg}_{pc}")
                    accB = ps.tile([D + 1, 512], F32, tag="avB", bufs=2,
                                   name=f"accB_{pg}_{pc}")
                    pend_norm = (pg, pc, accA, accB)
                nc.tensor.matmul(out=accA[:], lhsT=pvtA[:, pt_t, 0:D + 1],
                                 rhs=pptk[:, 0:512],
                                 start=(pt_t == 0), stop=(pt_t == KT - 1),
                                 skip_group_check=True)
                nc.tensor.matmul(out=accB[:], lhsT=pvtB[:, pt_t, 0:D + 1],
                                 rhs=pptk[:, 512:1024],
                                 start=(pt_t == 0), stop=(pt_t == KT - 1),
                                 skip_group_check=True)

        # drain the remaining LAG k-tiles, then the last norm
        for pptk, pt_t, pvtA, pvtB, pi in pend_av:
            nc.tensor.matmul(out=accA[:], lhsT=pvtA[:, pt_t, 0:D + 1],
                             rhs=pptk[:, 0:512],
                             start=(pt_t == 0), stop=(pt_t == KT - 1),
                             skip_group_check=True)
            nc.tensor.matmul(out=accB[:], lhsT=pvtB[:, pt_t, 0:D + 1],
                             rhs=pptk[:, 512:1024],
                             start=(pt_t == 0), stop=(pt_t == KT - 1),
                             skip_group_check=True)
        _emit_norm(nc, pools, pend_norm, aps)

    nc.compile()
    return nc


_NC = None


def _get_nc():
    global _NC
    if _NC is None:
        _NC = build()
    return _NC


def make_in_maps(q, k, v, k_length):
    q = np.ascontiguousarray(q, dtype=np.float32)
    k = np.ascontiguousarray(k, dtype=np.float32)
    v = np.ascontiguousarray(v, dtype=np.float32)
    k_length = np.asarray(k_length)
    in_maps = []
    for c in range(N_CORES):
        b = c // 2
        r0 = (c % 2) * ROWS
        in_maps.append({
            "q": np.ascontiguousarray(q[b, r0:r0 + ROWS, :]),
            "k": np.ascontiguousarray(k[b, r0:r0 + ROWS, :]),
            "v": np.ascontiguousarray(v[b, r0:r0 + ROWS, :]),
            "kl": np.array([[np.log(np.float32(k_length[b]))]],
                           dtype=np.float32),
        })
    return in_maps


def kernel(q, k, v, k_length):
    in_maps = make_in_maps(q, k, v, k_length)
    nc = _get_nc()
    res = run_bass_kernel_spmd(nc, in_maps, core_ids=list(range(N_CORES)))

    out = np.empty((B, H * D, L), dtype=np.float32)
    for c in range(N_CORES):
        b = c // 2
        r0 = (c % 2) * ROWS
        out[b, r0:r0 + ROWS, :] = res.results[c]["out"]
    return out
